# revision 41
# baseline (speedup 1.0000x reference)
"""MoE gate routing (nn_Gate) on 8 Trainium2 NeuronCores via Bass/Tile.

x: [32768, 2048] f32, weight: [64, 2048] f32.
Returns (weights [32768, 6] f32, indices [32768, 6] i32) matching
softmax(x @ W^T) -> top-6 (values sorted desc, ties -> lowest index).

Sharding: token-dim data parallel across 8 cores (4096 tokens/core);
the tiny gate weight is replicated. No collectives.

Per-core pipeline (8 slabs x 512 tokens):
  SWDGE DMA loads a slab of x -> PE transposes x into contraction-major
  layout (4 transposes packed per PSUM bank) -> wide DVE/ACT copies
  assemble the xT slab [128d, 16k, 512t] in SBUF -> 16 accumulating
  float32r matmuls (W^T chunks stationary, 512-wide moving operand at
  1 cyc/row) -> logitsT [64, 512] in PSUM -> ACT copy -> 4 small PE
  transposes back to token-major -> ACT Exp with fused per-token sum
  (accum_out) -> DVE reciprocal / max / max_index (native top-8) ->
  scale top-6 -> outputs accumulate in SBUF, two batched DMAs at the end.

Modeled (CoreSim cost model) at ~115 us/core, vs a ~102 us DMA floor
for the 33.5 MB/core of mandatory x traffic.
"""

import os
import numpy as np

N, DIM, E, TOPK = 32768, 2048, 64, 6
NCORES = 8
NTOK = N // NCORES          # 4096 tokens per core
SLAB = 512                  # tokens per slab (matmul moving dim)
NSLAB = NTOK // SLAB        # 8
TPS = SLAB // 128           # 4 token-tiles per slab
KCH = DIM // 128            # 16 contraction chunks

# matmul input dtype: "f32r" (full PE speed; ~1.4e-4 rel err on HW)
# or "f32" (exact fp32, 4x slower PE).
MM_DTYPE = os.environ.get("GATE_MM_DTYPE", "f32r")

_CACHE = {}
LAST_RESULTS = None


def _build_nc():
    import concourse.bass as bass
    import concourse.bacc as bacc
    import concourse.mybir as mybir
    from concourse import tile

    f32 = mybir.dt.float32
    f32r = mybir.dt.float32r
    i32 = mybir.dt.int32
    u32 = mybir.dt.uint32
    mm_dt = f32r if MM_DTYPE == "f32r" else f32

    nc = bacc.Bacc(None)

    x_d = nc.dram_tensor("x", [NTOK, DIM], f32, kind="ExternalInput")
    wt_d = nc.dram_tensor("wt", [128, KCH, E], f32, kind="ExternalInput")
    id_d = nc.dram_tensor("ident", [128, 128], f32, kind="ExternalInput")
    # packed output: per token 6 weights (f32 bits) then 8 top-8 indices
    out_d = nc.dram_tensor(
        "out", [NSLAB * TPS, 128, TOPK + 8], u32, kind="ExternalOutput"
    )

    Exp = mybir.ActivationFunctionType.Exp

    with tile.TileContext(nc) as tc:
        with (
            tc.tile_pool(name="const", bufs=1) as cpool,
            tc.tile_pool(name="xa", bufs=3) as xapool,
            tc.tile_pool(name="xt", bufs=2) as xtpool,
            tc.tile_pool(name="lsb", bufs=2) as lsbpool,
            tc.tile_pool(name="topk", bufs=4) as tkpool,
            tc.tile_pool(name="trp", bufs=4, space=bass.MemorySpace.PSUM) as trpool,
            tc.tile_pool(name="ltp", bufs=2, space=bass.MemorySpace.PSUM) as ltpool,
            tc.tile_pool(name="lgp", bufs=2, space=bass.MemorySpace.PSUM) as lgpool,
        ):
            wt_f32 = cpool.tile([128, KCH, E], f32, tag="wt_f32")
            nc.sync.dma_start(wt_f32[:], wt_d[:])
            ident = cpool.tile([128, 128], f32, tag="ident")
            nc.sync.dma_start(ident[:], id_d[:])
            if mm_dt is f32:
                wt_sb = wt_f32
            else:
                # round the gate weight to FP32r once on-chip
                wt_sb = cpool.tile([128, KCH, E], mm_dt, tag="wt_r")
                nc.scalar.copy(wt_sb[:], wt_f32[:])

            # all 32 tiles' outputs accumulate packed in SBUF; a single
            # batched DMA at the end
            ov_all = cpool.tile([128, NSLAB * TPS, TOPK + 8], u32, tag="ov_all")

            for s in range(NSLAB):
                xt = xtpool.tile([128, KCH, SLAB], mm_dt, tag="xt")
                # one whole-slab load per DMA
                xa = xapool.tile([128, TPS, DIM], f32, tag="xa")
                xsrc = x_d[s * SLAB : (s + 1) * SLAB, :].rearrange(
                    "(t p) d -> p t d", p=128
                )
                # per-tile DMAs: compute can start after 1/4 slab arrives
                for t in range(TPS):
                    nc.gpsimd.dma_start(xa[:, t], xsrc[:, t])
                # logitsT[e, tok] += wT_k.T @ xT_k accumulated over 16
                # chunks, as two independent 256-token halves so each half
                # completes (through top-k) while later tiles still load
                ltp = ltpool.tile([E, SLAB], f32, tag="ltp")
                lsb = lsbpool.tile([E, SLAB], f32, tag="lsb")
                half = SLAB // 2
                for t in range(TPS):
                    h, hi_tile = divmod(t, 2)
                    sl = slice(h * half, (h + 1) * half)
                    # 4 transposes per full PSUM bank, then one wide copy:
                    # amortizes the fixed per-instruction engine overheads.
                    for g in range(KCH // 4):
                        trp = trpool.tile([128, 4, 128], f32, tag="trp")
                        for j in range(4):
                            k = g * 4 + j
                            xsl = xa[:, t, k * 128 : (k + 1) * 128]
                            nc.tensor.matmul(
                                trp[:, j, :],
                                xsl,
                                ident[:],
                                is_transpose=True,
                                skip_group_check=True,
                            )
                        dst = xt[:, g * 4 : (g + 1) * 4, t * 128 : (t + 1) * 128]
                        if g % 2 == 0:
                            nc.scalar.copy(dst, trp[:])
                        else:
                            nc.vector.tensor_copy(dst, trp[:])
                    if not hi_tile:
                        continue
                    # dense 16-chunk matmul block per 256-token half, then
                    # the half's softmax/top-k
                    for k in range(KCH):
                        nc.tensor.matmul(
                            ltp[:, sl],
                            wt_sb[:, k, :],
                            xt[:, k, sl],
                            start=(k == 0),
                            stop=(k == KCH - 1),
                            skip_group_check=True,
                        )
                    nc.scalar.copy(lsb[:, sl], ltp[:, sl])
                    for tt in (t - 1, t):
                        # logits back to [token, expert] layout
                        lgp = lgpool.tile([128, E], f32, tag="lgp")
                        nc.tensor.transpose(
                            lgp[:],
                            lsb[:, tt * 128 : (tt + 1) * 128],
                            ident[:E, :E],
                        )
                        # p = exp(logits) with fused per-token sum (no
                        # max-subtraction needed: |logit| < ~6)
                        p = tkpool.tile([128, E], f32, tag="p")
                        psum_s = tkpool.tile([128, 1], f32, tag="psum_s")
                        nc.scalar.activation(
                            p[:], lgp[:], Exp, accum_out=psum_s[:]
                        )
                        rinv = tkpool.tile([128, 1], f32, tag="rinv")
                        nc.vector.reciprocal(rinv[:], psum_s[:])
                        v8 = tkpool.tile([128, 8], f32, tag="v8")
                        nc.vector.max(v8[:], p[:])
                        n = s * TPS + tt
                        nc.vector.max_index(ov_all[:, n, TOPK:], v8[:], p[:])
                        nc.vector.tensor_scalar_mul(
                            ov_all[:, n, :TOPK].bitcast(f32),
                            v8[:, :TOPK],
                            rinv[:],
                        )

            nc.gpsimd.dma_start(
                out_d.rearrange("n p c -> p n c"), ov_all[:]
            )

    # Bacc compile: splits >1-wait sync conditions into EventSemaphore
    # prefixes (HW allows 1 sync wait per instruction), register alloc, DCE.
    nc.compile()
    return nc


def _get_nc():
    if "nc" not in _CACHE:
        _CACHE["nc"] = _build_nc()
    return _CACHE["nc"]


def _host_inputs(weight):
    # wt[d, k, e] = weight[e, k*128 + d]
    wt = np.ascontiguousarray(
        weight.astype(np.float32).reshape(E, KCH, 128).transpose(2, 1, 0)
    )
    id_np = np.eye(128, dtype=np.float32)
    return wt, id_np


def kernel(x: np.ndarray, weight: np.ndarray):
    global LAST_RESULTS
    # the NTFF trace hook is unavailable under this axon client; a stray
    # BASS_TRACE=1 in the environment would crash run_bass_kernel_spmd
    os.environ["BASS_NEVER_TRACE"] = "1"
    from concourse.bass_utils import run_bass_kernel_spmd

    x = np.ascontiguousarray(np.asarray(x), dtype=np.float32)
    weight = np.asarray(weight)
    wt, id_np = _host_inputs(weight)

    nc = _get_nc()
    in_maps = [
        {"x": x[c * NTOK : (c + 1) * NTOK], "wt": wt, "ident": id_np}
        for c in range(NCORES)
    ]
    res = run_bass_kernel_spmd(nc, in_maps, list(range(NCORES)))
    LAST_RESULTS = res

    w_parts, i_parts = [], []
    for r in res.results:
        buf = np.ascontiguousarray(np.asarray(r["out"]).reshape(NTOK, TOPK + 8))
        w_parts.append(
            np.ascontiguousarray(buf[:, :TOPK]).view(np.float32)
        )
        i_parts.append(buf[:, TOPK : 2 * TOPK].astype(np.int32))
    weights_out = np.concatenate(w_parts, axis=0)
    indices_out = np.concatenate(i_parts, axis=0)
    return weights_out, indices_out



# revision 43
# speedup vs baseline: 1.0291x; 1.0291x over previous
"""MoE gate routing (nn_Gate) on 8 Trainium2 NeuronCores via Bass/Tile.

x: [32768, 2048] f32, weight: [64, 2048] f32.
Returns (weights [32768, 6] f32, indices [32768, 6] i32) matching
softmax(x @ W^T) -> top-6 (values sorted desc, ties -> lowest index).

Sharding: token-dim data parallel across 8 cores (4096 tokens/core);
the tiny gate weight is replicated. No collectives.

Per-core pipeline (8 slabs x 512 tokens):
  SWDGE DMA loads a slab of x -> PE transposes x into contraction-major
  layout (4 transposes packed per PSUM bank) -> wide DVE/ACT copies
  assemble the xT slab [128d, 16k, 512t] in SBUF -> 16 accumulating
  float32r matmuls (W^T chunks stationary, 512-wide moving operand at
  1 cyc/row) -> logitsT [64, 512] in PSUM -> ACT copy -> 4 small PE
  transposes back to token-major -> ACT Exp with fused per-token sum
  (accum_out) -> DVE reciprocal / max / max_index (native top-8) ->
  scale top-6 -> outputs accumulate in SBUF, two batched DMAs at the end.

Modeled (CoreSim cost model) at ~115 us/core, vs a ~102 us DMA floor
for the 33.5 MB/core of mandatory x traffic.
"""

import os
import numpy as np

N, DIM, E, TOPK = 32768, 2048, 64, 6
NCORES = 8
NTOK = N // NCORES          # 4096 tokens per core
SLAB = 512                  # tokens per slab (matmul moving dim)
NSLAB = NTOK // SLAB        # 8
TPS = SLAB // 128           # 4 token-tiles per slab
KCH = DIM // 128            # 16 contraction chunks

# matmul input dtype: "f32r" (full PE speed; ~1.4e-4 rel err on HW)
# or "f32" (exact fp32, 4x slower PE).
MM_DTYPE = os.environ.get("GATE_MM_DTYPE", "f32r")

_CACHE = {}
LAST_RESULTS = None


def _build_nc():
    import concourse.bass as bass
    import concourse.bacc as bacc
    import concourse.mybir as mybir
    from concourse import tile
    from concourse.tile import add_dep_helper

    f32 = mybir.dt.float32
    f32r = mybir.dt.float32r
    i32 = mybir.dt.int32
    u32 = mybir.dt.uint32
    mm_dt = f32r if MM_DTYPE == "f32r" else f32

    nc = bacc.Bacc(None)

    x_d = nc.dram_tensor("x", [NTOK, DIM], f32, kind="ExternalInput")
    wt_d = nc.dram_tensor("wt", [128, KCH, E], f32, kind="ExternalInput")
    id_d = nc.dram_tensor("ident", [128, 128], f32, kind="ExternalInput")
    # packed output: per token 6 weights (f32 bits) then 8 u16 indices
    out_d = nc.dram_tensor(
        "out", [NSLAB * TPS, 128, TOPK + 4], u32, kind="ExternalOutput"
    )

    Exp = mybir.ActivationFunctionType.Exp

    with tile.TileContext(nc) as tc:
        with (
            tc.tile_pool(name="const", bufs=1) as cpool,
            tc.tile_pool(name="xa", bufs=3) as xapool,
            tc.tile_pool(name="xt", bufs=2) as xtpool,
            tc.tile_pool(name="lsb", bufs=2) as lsbpool,
            tc.tile_pool(name="topk", bufs=4) as tkpool,
            tc.tile_pool(name="trp", bufs=4, space=bass.MemorySpace.PSUM) as trpool,
            tc.tile_pool(name="ltp", bufs=2, space=bass.MemorySpace.PSUM) as ltpool,
            tc.tile_pool(name="lgp", bufs=2, space=bass.MemorySpace.PSUM) as lgpool,
        ):
            wt_f32 = cpool.tile([128, KCH, E], f32, tag="wt_f32")
            nc.sync.dma_start(wt_f32[:], wt_d[:])
            ident = cpool.tile([128, 128], f32, tag="ident")
            nc.sync.dma_start(ident[:], id_d[:])
            if mm_dt is f32:
                wt_sb = wt_f32
            else:
                # round the gate weight to FP32r once on-chip
                wt_sb = cpool.tile([128, KCH, E], mm_dt, tag="wt_r")
                nc.scalar.copy(wt_sb[:], wt_f32[:])

            # all 32 tiles' outputs accumulate packed in SBUF; a single
            # batched DMA at the end
            ov_all = cpool.tile([128, NSLAB * TPS, TOPK + 4], u32, tag="ov_all")

            for s in range(NSLAB):
                xt = xtpool.tile([128, KCH, SLAB], mm_dt, tag="xt")
                # one whole-slab load per DMA
                xa = xapool.tile([128, TPS, DIM], f32, tag="xa")
                xsrc = x_d[s * SLAB : (s + 1) * SLAB, :].rearrange(
                    "(t p) d -> p t d", p=128
                )
                # per-tile DMAs: compute can start after 1/4 slab arrives
                for t in range(TPS):
                    last_xdma = nc.gpsimd.dma_start(xa[:, t], xsrc[:, t])
                # logitsT[e, tok] += wT_k.T @ xT_k accumulated over 16
                # chunks, as two independent 256-token halves so each half
                # completes (through top-k) while later tiles still load
                ltp = ltpool.tile([E, SLAB], f32, tag="ltp")
                lsb = lsbpool.tile([E, SLAB], f32, tag="lsb")
                half = SLAB // 2
                for t in range(TPS):
                    h, hi_tile = divmod(t, 2)
                    sl = slice(h * half, (h + 1) * half)
                    # 4 transposes per full PSUM bank, then one wide copy:
                    # amortizes the fixed per-instruction engine overheads.
                    for g in range(KCH // 4):
                        trp = trpool.tile([128, 4, 128], f32, tag="trp")
                        for j in range(4):
                            k = g * 4 + j
                            xsl = xa[:, t, k * 128 : (k + 1) * 128]
                            nc.tensor.matmul(
                                trp[:, j, :],
                                xsl,
                                ident[:],
                                is_transpose=True,
                                skip_group_check=True,
                            )
                        dst = xt[:, g * 4 : (g + 1) * 4, t * 128 : (t + 1) * 128]
                        if g % 2 == 0:
                            nc.scalar.copy(dst, trp[:])
                        else:
                            nc.vector.tensor_copy(dst, trp[:])
                    if not hi_tile:
                        continue
                    # dense 16-chunk matmul block per 256-token half, then
                    # the half's softmax/top-k
                    for k in range(KCH):
                        nc.tensor.matmul(
                            ltp[:, sl],
                            wt_sb[:, k, :],
                            xt[:, k, sl],
                            start=(k == 0),
                            stop=(k == KCH - 1),
                            skip_group_check=True,
                        )
                    nc.scalar.copy(lsb[:, sl], ltp[:, sl])
                    for tt in (t - 1, t):
                        # logits back to [token, expert] layout
                        lgp = lgpool.tile([128, E], f32, tag="lgp")
                        nc.tensor.transpose(
                            lgp[:],
                            lsb[:, tt * 128 : (tt + 1) * 128],
                            ident[:E, :E],
                        )
                        # p = exp(logits) with fused per-token sum (no
                        # max-subtraction needed: |logit| < ~6)
                        p = tkpool.tile([128, E], f32, tag="p")
                        psum_s = tkpool.tile([128, 1], f32, tag="psum_s")
                        nc.scalar.activation(
                            p[:], lgp[:], Exp, accum_out=psum_s[:]
                        )
                        rinv = tkpool.tile([128, 1], f32, tag="rinv")
                        nc.vector.reciprocal(rinv[:], psum_s[:])
                        v8 = tkpool.tile([128, 8], f32, tag="v8")
                        nc.vector.max(v8[:], p[:])
                        n = s * TPS + tt
                        u16 = mybir.dt.uint16
                        nc.vector.max_index(
                            ov_all[:, n, TOPK:].bitcast(u16), v8[:], p[:]
                        )
                        nc.vector.tensor_scalar_mul(
                            ov_all[:, n, :TOPK].bitcast(f32),
                            v8[:, :TOPK],
                            rinv[:],
                        )

            # flush slabs 0..6 during the last slab's compute epilogue:
            # ordered after the final x load so it can't delay the x stream
            nf = (NSLAB - 1) * TPS
            flush = nc.gpsimd.dma_start(
                out_d[:nf].rearrange("n p c -> p n c"), ov_all[:, :nf]
            )
            add_dep_helper(
                flush.ins,
                last_xdma.ins,
                reason="flush outputs only after the x stream finishes",
            )
            nc.gpsimd.dma_start(
                out_d[nf:].rearrange("n p c -> p n c"), ov_all[:, nf:]
            )

    # Bacc compile: splits >1-wait sync conditions into EventSemaphore
    # prefixes (HW allows 1 sync wait per instruction), register alloc, DCE.
    nc.compile()
    return nc


def _get_nc():
    if "nc" not in _CACHE:
        _CACHE["nc"] = _build_nc()
    return _CACHE["nc"]


def _host_inputs(weight):
    # wt[d, k, e] = weight[e, k*128 + d]
    wt = np.ascontiguousarray(
        weight.astype(np.float32).reshape(E, KCH, 128).transpose(2, 1, 0)
    )
    id_np = np.eye(128, dtype=np.float32)
    return wt, id_np


def kernel(x: np.ndarray, weight: np.ndarray):
    global LAST_RESULTS
    # the NTFF trace hook is unavailable under this axon client; a stray
    # BASS_TRACE=1 in the environment would crash run_bass_kernel_spmd
    os.environ["BASS_NEVER_TRACE"] = "1"
    from concourse.bass_utils import run_bass_kernel_spmd

    x = np.ascontiguousarray(np.asarray(x), dtype=np.float32)
    weight = np.asarray(weight)
    wt, id_np = _host_inputs(weight)

    nc = _get_nc()
    in_maps = [
        {"x": x[c * NTOK : (c + 1) * NTOK], "wt": wt, "ident": id_np}
        for c in range(NCORES)
    ]
    res = run_bass_kernel_spmd(nc, in_maps, list(range(NCORES)))
    LAST_RESULTS = res

    w_parts, i_parts = [], []
    for r in res.results:
        buf = np.ascontiguousarray(np.asarray(r["out"]).reshape(NTOK, TOPK + 4))
        w_parts.append(
            np.ascontiguousarray(buf[:, :TOPK]).view(np.float32)
        )
        iu16 = np.ascontiguousarray(buf[:, TOPK : TOPK + 3]).view(np.uint16)
        i_parts.append(iu16[:, :TOPK].astype(np.int32))
    weights_out = np.concatenate(w_parts, axis=0)
    indices_out = np.concatenate(i_parts, axis=0)
    return weights_out, indices_out

